# revision 14
# baseline (speedup 1.0000x reference)
"""Trainium2 Bass kernel for CombinedEmbedding.

reference: out[b,s,f] = W @ x[b,s,f] + pos_emb[s] + fmap_emb[f],
with x a one-hot [B,S,F,V] float32 tensor.

Strategy (8 NeuronCores, data-parallel over tokens):
  - flatten x to [16384 tokens, V=16384]; core c takes the contiguous
    2048-token slice (b = c//2, s in [32*(c%2), 32*(c%2)+32)).
  - per 128-token tile: DVE affine_mul_reduce (sum of x * iota == the
    one-hot index, exactly) recovers the token id. The DVE does ONLY
    these reduces + [P,1] index adds so it can always keep draining
    the x stream; the cast/gather/bias-add/store chain runs on GpSimd
    (except for the final tile, where it runs on the by-then-idle DVE
    to shorten the tail).
  - W^T is gathered in bf16 (1KB rows); pos_emb[s]+fmap_emb[f] is
    combined on the host into one [128, 16*512] bf16 bias tile that is
    DMAd into SBUF once; output is written bf16 and upcast on host.
  - x tiles stream as 4 MiB half-rows ping-ponged across the two HWDGE
    rings; the final tile streams as 4 quarter-rows to shorten the
    dependency tail after the last byte of x arrives.
"""

import numpy as np

B, S, F, V, E = 4, 64, 64, 16384, 512
NCORES = 8
TOKENS = B * S * F            # 16384
TPC = TOKENS // NCORES        # 2048 tokens per core
P = 128                       # partitions
NTILES = TPC // P             # 16 token tiles per core

_cache = {}


def _build():
    import concourse.bass as bass
    import concourse.bacc as bacc
    import concourse.mybir as mybir
    import concourse.tile as tile
    from concourse.alu_op_type import AluOpType

    nc = bacc.Bacc(trn_type="TRN2")
    x = nc.declare_dram_parameter("x", [TPC, V], mybir.dt.float32, isOutput=False)
    wt = nc.declare_dram_parameter("wt", [V, E], mybir.dt.bfloat16, isOutput=False)
    bias = nc.declare_dram_parameter("bias", [P, NTILES * E], mybir.dt.bfloat16,
                                     isOutput=False)
    out = nc.declare_dram_parameter("out", [TPC, E], mybir.dt.bfloat16, isOutput=True)

    # views
    x_t = x.rearrange("(t p) v -> t p v", p=P)               # [16,128,V]
    out_t = out.rearrange("(t p) e -> t p e", p=P)           # [16,128,E]
    wt_flat = wt[:, :]

    rings = [nc.sync, nc.scalar]  # the two HWDGE rings

    VH = V // 2
    VQ = V // 4
    LAST = NTILES - 1
    with tile.TileContext(nc) as tc:
        with (
            tc.tile_pool(name="px", bufs=3) as px,
            tc.tile_pool(name="pconst", bufs=1) as pconst,
            tc.tile_pool(name="pscr", bufs=4) as pscr,
            tc.tile_pool(name="pidx", bufs=1) as pidx,
            tc.tile_pool(name="pg", bufs=3) as pg,
        ):
            # iota table, generated in quarters so the first chunk is
            # ready before the first x half-tile lands.
            iota_sb = pconst.tile([P, V], mybir.dt.int16)
            bias_sb = pconst.tile([P, NTILES * E], mybir.dt.bfloat16)
            rings[0].dma_start(out=bias_sb[:, :], in_=bias[:, :])
            for q in range(4):
                nc.gpsimd.iota(
                    iota_sb[:, q * VQ:(q + 1) * VQ],
                    pattern=[[1, VQ]], base=q * VQ, channel_multiplier=0,
                )

            idx_all = pidx.tile([P, NTILES], mybir.dt.float32)
            idx_i = pidx.tile([P, NTILES], mybir.dt.int32)
            dummy = pidx.tile([P, 1], mybir.dt.float32)

            def reduce_chunk(t, tag, lo, width, acc_ap):
                xt = px.tile([P, width], mybir.dt.float32, tag=tag)
                rings[(2 * t + lo // width) % 2].dma_start(
                    out=xt[:, :], in_=x_t[t, :, lo:lo + width]
                )
                # one-hot: sum(x * iota) over the chunk == idx or 0.
                nc.vector.affine_mul_reduce(
                    out=dummy.broadcast_to((P, width)),
                    accum_out=acc_ap,
                    in0=xt[:, :],
                    in1=iota_sb[:, lo:lo + width],
                    scale=1.0,
                    bias=0.0,
                )

            def tail_chain(t):
                # While x is still streaming, the cast/add run on Q7:
                # the slow Q7 vector ops space out the SWDGE dispatches,
                # whose clustered random-row descriptor bursts would
                # otherwise starve the x stream (measured: 20 -> 26.5
                # us/tile).  The last four tiles' chains run on the DVE
                # instead (~0.6us vs ~9us per tile), which drains the
                # post-stream tail ~4x faster.  ALL outs stay on SWDGE:
                # an out on a HWDGE ring queues behind its compute dep
                # and head-of-line blocks later x DMAs (measured 31.5us
                # stall).
                drain = t >= NTILES - 4
                veng = nc.vector if drain else nc.gpsimd
                veng.tensor_copy(idx_i[:, t:t + 1], idx_all[:, t:t + 1])
                gath = pg.tile([P, E], mybir.dt.bfloat16, tag="gath")
                nc.gpsimd.indirect_dma_start(
                    out=gath[:, :],
                    out_offset=None,
                    in_=wt_flat,
                    in_offset=bass.IndirectOffsetOnAxis(
                        ap=idx_i[:, t:t + 1], axis=0
                    ),
                )
                outg = pg.tile([P, E], mybir.dt.bfloat16, tag="outg")
                veng.tensor_tensor(
                    out=outg[:, :],
                    in0=gath[:, :],
                    in1=bias_sb[:, t * E:(t + 1) * E],
                    op=AluOpType.add,
                )
                nc.gpsimd.dma_start(out=out_t[t], in_=outg[:, :])

            for t in range(LAST):
                idx_tmp = pscr.tile([P, 2], mybir.dt.float32, tag="idx_tmp")
                for h in range(2):
                    reduce_chunk(t, "xh", h * VH, VH, idx_tmp[:, h:h + 1])
                nc.vector.tensor_add(
                    out=idx_all[:, t:t + 1],
                    in0=idx_tmp[:, 0:1],
                    in1=idx_tmp[:, 1:2],
                )
                tail_chain(t)

            # last tile in quarters: shortens the post-last-byte chain
            t = LAST
            idx_q = pscr.tile([P, 4], mybir.dt.float32, tag="idx_q")
            for q in range(4):
                reduce_chunk(t, "xq", q * VQ, VQ, idx_q[:, q:q + 1])
            qsum = pscr.tile([P, 2], mybir.dt.float32, tag="qsum")
            nc.vector.tensor_add(out=qsum[:, 0:1], in0=idx_q[:, 0:1], in1=idx_q[:, 1:2])
            nc.vector.tensor_add(out=qsum[:, 1:2], in0=idx_q[:, 2:3], in1=idx_q[:, 3:4])
            nc.vector.tensor_add(
                out=idx_all[:, t:t + 1], in0=qsum[:, 0:1], in1=qsum[:, 1:2]
            )
            tail_chain(t)
    nc.finalize()
    return nc


def _host_shards(x, W, pos_emb, fmap_emb):
    import ml_dtypes

    bf16 = ml_dtypes.bfloat16
    x_flat = np.ascontiguousarray(x.reshape(TOKENS, V))
    wt = np.ascontiguousarray(W.T).astype(bf16)         # [V, E] bf16

    fmap_part = fmap_emb[np.arange(P) % F]              # [128, E]
    in_maps = []
    for c in range(NCORES):
        s_base = (c % (S // 32)) * 32
        # bias[p, t*E + e] = pos_emb[s_base + 2t + p//64, e] + fmap_emb[p%64, e]
        s_idx = s_base + 2 * np.arange(NTILES)[None, :] + (np.arange(P) // F)[:, None]
        bias = pos_emb[s_idx] + fmap_part[:, None, :]   # [128, 16, E]
        in_maps.append({
            "x": x_flat[c * TPC:(c + 1) * TPC],
            "wt": wt,
            "bias": np.ascontiguousarray(bias.reshape(P, NTILES * E).astype(bf16)),
        })
    return in_maps


def kernel(x, W, pos_emb, fmap_emb):
    from concourse import bass_utils

    x = np.asarray(x, dtype=np.float32)
    W = np.asarray(W, dtype=np.float32)
    pos_emb = np.asarray(pos_emb, dtype=np.float32)
    fmap_emb = np.asarray(fmap_emb, dtype=np.float32)

    if "nc" not in _cache:
        _cache["nc"] = _build()
    nc = _cache["nc"]

    in_maps = _host_shards(x, W, pos_emb, fmap_emb)
    res = bass_utils.run_bass_kernel_spmd(nc, in_maps, core_ids=list(range(NCORES)))
    outs = [np.asarray(res.results[c]["out"]).astype(np.float32)
            for c in range(NCORES)]
    full = np.concatenate(outs, axis=0).reshape(B, S, F, E)
    return full
